# revision 6
# baseline (speedup 1.0000x reference)
import numpy as np

import concourse.bass as bass
import concourse.bacc as bacc
import concourse.mybir as mybir
import concourse.tile as tile
from concourse.bass_utils import run_bass_kernel_spmd

B, T, C, H, D = 2, 2048, 1024, 16, 64
NCORES = 8
HPC = 4
CL = HPC * D
TS = 512
NTB = T // 128
NKC = C // 128
SM_SCALE = 1.0 / 8.0
GROUPS = [[0, 1, 2, 3], [4, 5, 6, 7]]

f32 = mybir.dt.float32
bf16 = mybir.dt.bfloat16
Exp = mybir.ActivationFunctionType.Exp

TRACE = False
TRACE_KWARGS = {}
LAST_RESULTS = None

_cached_nc = None


def _emit(nc, tc):
    xT_ext = nc.dram_tensor("xT", [C, T], bf16, kind="ExternalInput")
    wqk_ext = nc.dram_tensor("wqk", [C, 2 * CL], bf16, kind="ExternalInput")
    wv_ext = nc.dram_tensor("wv", [C, CL], bf16, kind="ExternalInput")
    wp_ext = nc.dram_tensor("wp", [CL, C], bf16, kind="ExternalInput")
    bqk_ext = nc.dram_tensor("bqk", [1, 2 * CL], bf16, kind="ExternalInput")
    bv_ext = nc.dram_tensor("bv", [1, CL], bf16, kind="ExternalInput")
    out_ext = nc.dram_tensor("out", [TS, C], f32, kind="ExternalOutput")
    pbounce = nc.dram_tensor("pbounce", [T, C], bf16)
    rs_out = nc.dram_tensor("rs_out", [TS, C], bf16)

    with tc.tile_pool(name="persist", bufs=1) as pp, \
         tc.tile_pool(name="work", bufs=3) as wk, \
         tc.tile_pool(name="psum", bufs=2, space="PSUM") as psum:

        bqk = pp.tile([1, 2 * CL], bf16, tag="bqk")
        nc.sync.dma_start(out=bqk[:], in_=bqk_ext[:])
        bv = pp.tile([1, CL], bf16, tag="bv")
        nc.sync.dma_start(out=bv[:], in_=bv_ext[:])
        ones = pp.tile([1, TS], bf16, tag="ones")
        nc.gpsimd.memset(ones[:], 1.0)

        xts, wqk, wv = [], [], []
        for kc in range(NKC):
            tw = pp.tile([128, 2 * CL], bf16, tag=f"wqk{kc}", name=f"wqk{kc}")
            nc.sync.dma_start(out=tw[:], in_=wqk_ext[128 * kc:128 * (kc + 1), :])
            wqk.append(tw)
            tx = pp.tile([128, T], bf16, tag=f"xt{kc}", name=f"xt{kc}")
            nc.sync.dma_start(out=tx[:], in_=xT_ext[128 * kc:128 * (kc + 1), :])
            xts.append(tx)
        for kc in range(NKC):
            t_ = pp.tile([128, CL], bf16, tag=f"wv{kc}", name=f"wv{kc}")
            nc.sync.dma_start(out=t_[:], in_=wv_ext[128 * kc:128 * (kc + 1), :])
            wv.append(t_)
        wp = []
        for kb in range(2):
            t_ = pp.tile([128, C], bf16, tag=f"wp{kb}", name=f"wp{kb}")
            nc.sync.dma_start(out=t_[:], in_=wp_ext[128 * kb:128 * (kb + 1), :])
            wp.append(t_)

        qk_sb = [pp.tile([128, T], bf16, tag=f"qk{mb}", name=f"qk{mb}")
                 for mb in range(4)]
        va = [pp.tile([128, HPC * (D + 1)], bf16, tag=f"va{tb}", name=f"va{tb}")
              for tb in range(NTB)]
        yn = [pp.tile([128, T], bf16, tag=f"yn{kb}", name=f"yn{kb}")
              for kb in range(2)]

        for mb in range(4):
            for s in range(4):
                pt = psum.tile([128, TS], f32, tag="mm")
                for kc in range(NKC):
                    nc.tensor.matmul(
                        pt[:],
                        lhsT=wqk[kc][:, 128 * mb:128 * (mb + 1)],
                        rhs=xts[kc][:, TS * s:TS * (s + 1)],
                        start=(kc == 0), stop=False)
                nc.tensor.matmul(
                    pt[:],
                    lhsT=bqk[0:1, 128 * mb:128 * (mb + 1)],
                    rhs=ones[0:1, :],
                    start=False, stop=True)
                nc.vector.tensor_copy(qk_sb[mb][:, TS * s:TS * (s + 1)], pt[:])

        for tb in range(NTB):
            for h in range(HPC):
                nc.gpsimd.memset(va[tb][:, 65 * h + 64:65 * h + 65], 1.0)
            pv = psum.tile([128, TS], f32, tag="mm")
            for kc in range(NKC):
                nc.tensor.matmul(
                    pv[:, 0:CL],
                    lhsT=xts[kc][:, 128 * tb:128 * (tb + 1)],
                    rhs=wv[kc][:],
                    start=(kc == 0), stop=False)
            nc.tensor.matmul(
                pv[:, 0:CL],
                lhsT=ones[0:1, 0:128],
                rhs=bv[0:1, :],
                start=False, stop=True)
            dst = va[tb][:].rearrange("p (h e) -> p h e", e=D + 1)[:, :, 0:D]
            src = pv[:, 0:CL].rearrange("p (h d) -> p h d", d=D)
            nc.vector.tensor_copy(dst, src)

        for s in range(4):
            nkb = 4 * s + 4
            for h in range(HPC):
                po = 64 * (h % 2)
                qh = qk_sb[h // 2][po:po + 64, :]
                kh = qk_sb[2 + h // 2][po:po + 64, :]
                yt = psum.tile([D + 1, TS], f32, tag="yt")
                for kb0 in range(0, nkb, 2):
                    nblk = min(2, nkb - kb0)
                    st = psum.tile([128, 2 * TS], f32, tag="st", bufs=2)
                    for j in range(nblk):
                        kb = kb0 + j
                        nc.tensor.matmul(
                            st[:, TS * j:TS * (j + 1)],
                            lhsT=kh[:, 128 * kb:128 * (kb + 1)],
                            rhs=qh[:, TS * s:TS * (s + 1)],
                            start=True, stop=True)
                    est = wk.tile([128, 2 * TS], bf16, tag="est", bufs=4)
                    nc.scalar.activation(
                        est[:, :TS * nblk], st[:, :TS * nblk], Exp,
                        scale=SM_SCALE)
                    for j in range(nblk):
                        kb = kb0 + j
                        if kb >= 4 * s:
                            nc.gpsimd.affine_select(
                                out=est[:, TS * j:TS * (j + 1)],
                                in_=est[:, TS * j:TS * (j + 1)],
                                compare_op=mybir.AluOpType.is_ge,
                                fill=0.0,
                                base=TS * s - 128 * kb,
                                channel_multiplier=-1,
                                pattern=[[1, TS]])
                        nc.tensor.matmul(
                            yt[:],
                            lhsT=va[kb][:, 65 * h:65 * h + 65],
                            rhs=est[:, TS * j:TS * (j + 1)],
                            start=(kb == 0), stop=(kb == nkb - 1))
                rc = wk.tile([1, TS], f32, tag="rc")
                nc.vector.reciprocal(rc[:], yt[D:D + 1, :])
                bc = wk.tile([D, TS], f32, tag="bc")
                nc.gpsimd.partition_broadcast(bc[:], rc[:])
                nc.vector.tensor_mul(
                    yn[h // 2][po:po + 64, TS * s:TS * (s + 1)],
                    yt[0:D, :], bc[:])

            for tbl in range(4):
                tb = 4 * s + tbl
                for half in range(2):
                    pj = psum.tile([128, TS], f32, tag="mm")
                    for kb in range(2):
                        nc.tensor.matmul(
                            pj[:],
                            lhsT=yn[kb][:, 128 * tb:128 * (tb + 1)],
                            rhs=wp[kb][:, TS * half:TS * (half + 1)],
                            start=(kb == 0), stop=(kb == 1))
                    ps = wk.tile([128, TS], bf16, tag="po")
                    nc.vector.tensor_copy(ps[:], pj[:])
                    nc.sync.dma_start(
                        out=pbounce[128 * tb:128 * (tb + 1),
                                    TS * half:TS * (half + 1)],
                        in_=ps[:])
            nc.gpsimd.collective_compute(
                "ReduceScatter",
                mybir.AluOpType.add,
                replica_groups=GROUPS,
                ins=[pbounce[TS * s:TS * (s + 1), :]],
                outs=[rs_out[128 * s:128 * (s + 1), :]],
            )
            us = wk.tile([128, C], bf16, tag="us")
            nc.sync.dma_start(out=us[:], in_=rs_out[128 * s:128 * (s + 1), :])
            uf = wk.tile([128, C], f32, tag="uf")
            nc.vector.tensor_copy(uf[:], us[:])
            nc.sync.dma_start(out=out_ext[128 * s:128 * (s + 1), :], in_=uf[:])


def _build():
    global _cached_nc
    if _cached_nc is not None:
        return _cached_nc
    nc = bacc.Bacc("TRN2", target_bir_lowering=False, debug=False,
                   num_devices=NCORES)
    with tile.TileContext(nc) as tc:
        _emit(nc, tc)
    nc.compile()
    _cached_nc = nc
    return nc


def kernel(x, w_attn, b_attn, w_proj, b_proj):
    global LAST_RESULTS
    nc = _build()
    np_bf16 = mybir.dt.np(bf16)

    x = np.asarray(x, np.float32)
    w_attn = np.asarray(w_attn, np.float32)
    b_attn = np.asarray(b_attn, np.float32)
    w_proj = np.asarray(w_proj, np.float32)
    b_proj = np.asarray(b_proj, np.float32)

    xT = [np.ascontiguousarray(x[b].T).astype(np_bf16) for b in range(B)]

    in_maps = []
    for c in range(NCORES):
        b = c // 4
        g = c % 4
        cols = slice(CL * g, CL * (g + 1))
        wqk_c = np.concatenate(
            [w_attn[:, cols], w_attn[:, C + CL * g:C + CL * (g + 1)]],
            axis=1).astype(np_bf16)
        wv_c = np.ascontiguousarray(
            w_attn[:, 2 * C + CL * g:2 * C + CL * (g + 1)]).astype(np_bf16)
        wp_c = np.ascontiguousarray(w_proj[cols, :]).astype(np_bf16)
        bqk_c = np.concatenate(
            [b_attn[cols], b_attn[C + CL * g:C + CL * (g + 1)]]
        ).reshape(1, -1).astype(np_bf16)
        bv_c = b_attn[2 * C + CL * g:2 * C + CL * (g + 1)].reshape(
            1, -1).astype(np_bf16)
        in_maps.append({
            "xT": xT[b], "wqk": wqk_c, "wv": wv_c, "wp": wp_c,
            "bqk": bqk_c, "bv": bv_c,
        })

    res = run_bass_kernel_spmd(nc, in_maps, core_ids=list(range(NCORES)),
                               trace=TRACE, **TRACE_KWARGS)
    LAST_RESULTS = res

    y = np.empty((B, T, C), np.float32)
    for b in range(B):
        for r in range(4):
            shard = res.results[4 * b + r]["out"]
            for s in range(4):
                y[b][TS * s + 128 * r: TS * s + 128 * (r + 1)] = \
                    shard[128 * s:128 * (s + 1)]
    y += b_proj[None, None, :]
    return y


# revision 8
# speedup vs baseline: 1.4878x; 1.4878x over previous
import numpy as np

import concourse.bass as bass
import concourse.bacc as bacc
import concourse.mybir as mybir
import concourse.tile as tile
from concourse.bass_utils import run_bass_kernel_spmd

B, T, C, H, D = 2, 2048, 1024, 16, 64
NCORES = 8
HPC = 4
CL = HPC * D
TS = 512
NTB = T // 128
NKC = C // 128
SM_SCALE = 1.0 / 8.0
GROUPS = [[0, 1, 2, 3], [4, 5, 6, 7]]

f32 = mybir.dt.float32
bf16 = mybir.dt.bfloat16
Exp = mybir.ActivationFunctionType.Exp

TRACE = False
TRACE_KWARGS = {}
LAST_RESULTS = None

_cached_nc = None


def _emit(nc, tc):
    xT_ext = nc.dram_tensor("xT", [C, T], bf16, kind="ExternalInput")
    wqk_ext = nc.dram_tensor("wqk", [C, 2 * CL], bf16, kind="ExternalInput")
    wv_ext = nc.dram_tensor("wv", [C, CL], bf16, kind="ExternalInput")
    wp_ext = nc.dram_tensor("wp", [CL, C], bf16, kind="ExternalInput")
    bqk_ext = nc.dram_tensor("bqk", [1, 2 * CL], bf16, kind="ExternalInput")
    bv_ext = nc.dram_tensor("bv", [1, CL], bf16, kind="ExternalInput")
    out_ext = nc.dram_tensor("out", [TS, C], f32, kind="ExternalOutput")
    pb = [nc.dram_tensor(f"pb{s}", [TS, C], bf16) for s in range(4)]
    rs = [nc.dram_tensor(f"rs{s}", [128, C], bf16) for s in range(4)]
    warm_in = nc.dram_tensor("warm_in", [1, 128], bf16)
    warm_out = nc.dram_tensor("warm_out", [1, 128], bf16)

    with tc.tile_pool(name="persist", bufs=1) as pp, \
         tc.tile_pool(name="work", bufs=3) as wk, \
         tc.tile_pool(name="psum", bufs=2, space="PSUM") as psum:

        wt = pp.tile([1, 128], bf16, tag="wt")
        nc.gpsimd.memset(wt[:], 0.0)
        nc.sync.dma_start(out=warm_in[:], in_=wt[:])
        nc.gpsimd.collective_compute(
            "AllReduce", mybir.AluOpType.add, replica_groups=GROUPS,
            ins=[warm_in[:]], outs=[warm_out[:]])

        bqk = pp.tile([1, 2 * CL], bf16, tag="bqk")
        nc.sync.dma_start(out=bqk[:], in_=bqk_ext[:])
        bv = pp.tile([1, CL], bf16, tag="bv")
        nc.sync.dma_start(out=bv[:], in_=bv_ext[:])
        ones = pp.tile([1, TS], bf16, tag="ones")
        nc.gpsimd.memset(ones[:], 1.0)

        xts, wqk, wv = [], [], []
        for kc in range(NKC):
            tw = pp.tile([128, 2 * CL], bf16, tag=f"wqk{kc}", name=f"wqk{kc}")
            nc.sync.dma_start(out=tw[:], in_=wqk_ext[128 * kc:128 * (kc + 1), :])
            wqk.append(tw)
            tx = pp.tile([128, T], bf16, tag=f"xt{kc}", name=f"xt{kc}")
            nc.sync.dma_start(out=tx[:], in_=xT_ext[128 * kc:128 * (kc + 1), :])
            xts.append(tx)
        for kc in range(NKC):
            t_ = pp.tile([128, CL], bf16, tag=f"wv{kc}", name=f"wv{kc}")
            nc.sync.dma_start(out=t_[:], in_=wv_ext[128 * kc:128 * (kc + 1), :])
            wv.append(t_)
        wp = []
        for kb in range(2):
            t_ = pp.tile([128, C], bf16, tag=f"wp{kb}", name=f"wp{kb}")
            nc.sync.dma_start(out=t_[:], in_=wp_ext[128 * kb:128 * (kb + 1), :])
            wp.append(t_)

        qk_sb = [pp.tile([128, T], bf16, tag=f"qk{mb}", name=f"qk{mb}")
                 for mb in range(4)]
        va = [pp.tile([128, HPC * (D + 1)], bf16, tag=f"va{tb}", name=f"va{tb}")
              for tb in range(NTB)]
        yn = [pp.tile([128, T], bf16, tag=f"yn{kb}", name=f"yn{kb}")
              for kb in range(2)]

        for mb in range(4):
            for s in range(4):
                pt = psum.tile([128, TS], f32, tag="mm")
                for kc in range(NKC):
                    nc.tensor.matmul(
                        pt[:],
                        lhsT=wqk[kc][:, 128 * mb:128 * (mb + 1)],
                        rhs=xts[kc][:, TS * s:TS * (s + 1)],
                        start=(kc == 0), stop=False)
                nc.tensor.matmul(
                    pt[:],
                    lhsT=bqk[0:1, 128 * mb:128 * (mb + 1)],
                    rhs=ones[0:1, :],
                    start=False, stop=True)
                nc.vector.tensor_copy(qk_sb[mb][:, TS * s:TS * (s + 1)], pt[:])

        for tb in range(NTB):
            for h in range(HPC):
                nc.gpsimd.memset(va[tb][:, 65 * h + 64:65 * h + 65], 1.0)
            pv = psum.tile([128, TS], f32, tag="mm")
            for kc in range(NKC):
                nc.tensor.matmul(
                    pv[:, 0:CL],
                    lhsT=xts[kc][:, 128 * tb:128 * (tb + 1)],
                    rhs=wv[kc][:],
                    start=(kc == 0), stop=False)
            nc.tensor.matmul(
                pv[:, 0:CL],
                lhsT=ones[0:1, 0:128],
                rhs=bv[0:1, :],
                start=False, stop=True)
            dst = va[tb][:].rearrange("p (h e) -> p h e", e=D + 1)[:, :, 0:D]
            src = pv[:, 0:CL].rearrange("p (h d) -> p h d", d=D)
            nc.vector.tensor_copy(dst, src)

        for s in range(4):
            nkb = 4 * s + 4
            for h in range(HPC):
                po = 64 * (h % 2)
                qh = qk_sb[h // 2][po:po + 64, :]
                kh = qk_sb[2 + h // 2][po:po + 64, :]
                yt = psum.tile([D + 1, TS], f32, tag="yt")
                for kb0 in range(0, nkb, 2):
                    nblk = min(2, nkb - kb0)
                    st = psum.tile([128, 2 * TS], f32, tag="st", bufs=2)
                    for j in range(nblk):
                        kb = kb0 + j
                        nc.tensor.matmul(
                            st[:, TS * j:TS * (j + 1)],
                            lhsT=kh[:, 128 * kb:128 * (kb + 1)],
                            rhs=qh[:, TS * s:TS * (s + 1)],
                            start=True, stop=True)
                    est = wk.tile([128, 2 * TS], bf16, tag="est", bufs=4)
                    nc.scalar.activation(
                        est[:, :TS * nblk], st[:, :TS * nblk], Exp,
                        scale=SM_SCALE)
                    for j in range(nblk):
                        kb = kb0 + j
                        if kb >= 4 * s:
                            nc.gpsimd.affine_select(
                                out=est[:, TS * j:TS * (j + 1)],
                                in_=est[:, TS * j:TS * (j + 1)],
                                compare_op=mybir.AluOpType.is_ge,
                                fill=0.0,
                                base=TS * s - 128 * kb,
                                channel_multiplier=-1,
                                pattern=[[1, TS]])
                        nc.tensor.matmul(
                            yt[:],
                            lhsT=va[kb][:, 65 * h:65 * h + 65],
                            rhs=est[:, TS * j:TS * (j + 1)],
                            start=(kb == 0), stop=(kb == nkb - 1))
                rs_sb = wk.tile([1, TS], f32, tag="rs_sb")
                nc.vector.tensor_copy(rs_sb[:], yt[D:D + 1, :])
                rc = wk.tile([1, TS], f32, tag="rc")
                nc.vector.reciprocal_approx_fast(rc[:], rs_sb[:])
                bc = wk.tile([D, TS], f32, tag="bc")
                nc.gpsimd.partition_broadcast(bc[:], rc[:])
                nc.vector.tensor_mul(
                    yn[h // 2][po:po + 64, TS * s:TS * (s + 1)],
                    yt[0:D, :], bc[:])

            for tbl in range(4):
                tb = 4 * s + tbl
                for half in range(2):
                    pj = psum.tile([128, TS], f32, tag="mm")
                    for kb in range(2):
                        nc.tensor.matmul(
                            pj[:],
                            lhsT=yn[kb][:, 128 * tb:128 * (tb + 1)],
                            rhs=wp[kb][:, TS * half:TS * (half + 1)],
                            start=(kb == 0), stop=(kb == 1))
                    ps = wk.tile([128, TS], bf16, tag="po")
                    nc.vector.tensor_copy(ps[:], pj[:])
                    nc.sync.dma_start(
                        out=pb[s][128 * tbl:128 * (tbl + 1),
                                  TS * half:TS * (half + 1)],
                        in_=ps[:])
            nc.gpsimd.collective_compute(
                "ReduceScatter",
                mybir.AluOpType.add,
                replica_groups=GROUPS,
                ins=[pb[s][:]],
                outs=[rs[s][:]],
            )

        for s in range(4):
            us = wk.tile([128, C], bf16, tag="us", bufs=2)
            nc.sync.dma_start(out=us[:], in_=rs[s][:])
            uf = wk.tile([128, C], f32, tag="uf", bufs=2)
            nc.vector.tensor_copy(uf[:], us[:])
            nc.sync.dma_start(out=out_ext[128 * s:128 * (s + 1), :], in_=uf[:])


def _build():
    global _cached_nc
    if _cached_nc is not None:
        return _cached_nc
    nc = bacc.Bacc("TRN2", target_bir_lowering=False, debug=False,
                   num_devices=NCORES)
    with tile.TileContext(nc) as tc:
        _emit(nc, tc)
    nc.compile()
    _cached_nc = nc
    return nc


def kernel(x, w_attn, b_attn, w_proj, b_proj):
    global LAST_RESULTS
    nc = _build()
    np_bf16 = mybir.dt.np(bf16)

    x = np.asarray(x, np.float32)
    w_attn = np.asarray(w_attn, np.float32)
    b_attn = np.asarray(b_attn, np.float32)
    w_proj = np.asarray(w_proj, np.float32)
    b_proj = np.asarray(b_proj, np.float32)

    xT = [np.ascontiguousarray(x[b].T).astype(np_bf16) for b in range(B)]

    in_maps = []
    for c in range(NCORES):
        b = c // 4
        g = c % 4
        cols = slice(CL * g, CL * (g + 1))
        wqk_c = np.concatenate(
            [w_attn[:, cols], w_attn[:, C + CL * g:C + CL * (g + 1)]],
            axis=1).astype(np_bf16)
        wv_c = np.ascontiguousarray(
            w_attn[:, 2 * C + CL * g:2 * C + CL * (g + 1)]).astype(np_bf16)
        wp_c = np.ascontiguousarray(w_proj[cols, :]).astype(np_bf16)
        bqk_c = np.concatenate(
            [b_attn[cols], b_attn[C + CL * g:C + CL * (g + 1)]]
        ).reshape(1, -1).astype(np_bf16)
        bv_c = b_attn[2 * C + CL * g:2 * C + CL * (g + 1)].reshape(
            1, -1).astype(np_bf16)
        in_maps.append({
            "xT": xT[b], "wqk": wqk_c, "wv": wv_c, "wp": wp_c,
            "bqk": bqk_c, "bv": bv_c,
        })

    res = run_bass_kernel_spmd(nc, in_maps, core_ids=list(range(NCORES)),
                               trace=TRACE, **TRACE_KWARGS)
    LAST_RESULTS = res

    y = np.empty((B, T, C), np.float32)
    for b in range(B):
        for r in range(4):
            shard = res.results[4 * b + r]["out"]
            for s in range(4):
                y[b][TS * s + 128 * r: TS * s + 128 * (r + 1)] = \
                    shard[128 * s:128 * (s + 1)]
    y += b_proj[None, None, :]
    return y
